# revision 1
# baseline (speedup 1.0000x reference)
"""Tree-GRU classifier on 8 Trainium2 NeuronCores.

Strategy (per sharding hint): data-parallel over batch B=64 -> 8 per core,
small weights (embedding table, GRU/linear params) replicated on every core.
Tree aggregation and the GRU scans are independent per sample, so there is
no cross-core communication; the full output is gathered on host.
The model is compiled for the Neuron cores through PJRT and executed SPMD
with jax.pmap.
"""
import numpy as np
import jax
import jax.numpy as jnp

LEVELS = 5
N_NODES = 2 ** LEVELS - 1          # 31
V, E, ENC, H, LBL = 50000, 128, 128, 128, 104
B, L = 64, 128
N_CORES = 8

WEIGHT_KEYS = [
    "embedding", "Wc_w", "Wc_b",
    "Wih_f", "Whh_f", "bih_f", "bhh_f",
    "Wih_b", "Whh_b", "bih_b", "bhh_b",
    "Wout", "bout",
]


def _model(tokens, embedding, Wc_w, Wc_b,
           Wih_f, Whh_f, bih_f, bhh_f,
           Wih_b, Whh_b, bih_b, bhh_b,
           Wout, bout):
    # tokens: [b, L, N_NODES] int32 (per-core batch shard)
    x = embedding[tokens]                                  # [b, L, N, E]
    h = jnp.einsum("blne,ce->blnc", x, Wc_w) + Wc_b        # [b, L, N, ENC]
    for lvl in reversed(range(LEVELS - 1)):                # bottom-up tree sum
        s = 2 ** lvl - 1
        n = 2 ** lvl
        cs = 2 * s + 1
        left = h[..., cs:cs + 2 * n:2, :]
        right = h[..., cs + 1:cs + 2 * n:2, :]
        h = h.at[..., s:s + n, :].add(left + right)
    enc = jnp.max(h, axis=2)                               # [b, L, ENC]

    def gru_dir(xs, Wih, Whh, bih, bhh, reverse):
        gi = jnp.einsum("blc,gc->blg", xs, Wih) + bih      # [b, L, 3H]

        def step(hh, gi_t):
            gh = hh @ Whh.T + bhh
            ir, iz, inn = jnp.split(gi_t, 3, axis=-1)
            hr, hz, hn = jnp.split(gh, 3, axis=-1)
            r = jax.nn.sigmoid(ir + hr)
            z = jax.nn.sigmoid(iz + hz)
            nn_ = jnp.tanh(inn + r * hn)
            h_new = (1.0 - z) * nn_ + z * hh
            return h_new, h_new

        h0 = jnp.zeros((xs.shape[0], H), dtype=xs.dtype)
        _, ys = jax.lax.scan(step, h0, gi.transpose(1, 0, 2), reverse=reverse)
        return ys.transpose(1, 0, 2)                       # [b, L, H]

    fwd = gru_dir(enc, Wih_f, Whh_f, bih_f, bhh_f, False)
    bwd = gru_dir(enc, Wih_b, Whh_b, bih_b, bhh_b, True)
    gru_out = jnp.concatenate([fwd, bwd], axis=-1)         # [b, L, 2H]
    pooled = jnp.max(gru_out, axis=1)                      # [b, 2H]
    return pooled @ Wout.T + bout                          # [b, LBL]


_pmodel = jax.pmap(_model, in_axes=(0,) + (None,) * len(WEIGHT_KEYS))


def kernel(**inputs) -> np.ndarray:
    tokens = np.asarray(inputs["tokens"])
    b_total = tokens.shape[0]
    # V=50000 fits int32; Neuron runs with 32-bit index types.
    tok_sharded = tokens.reshape(N_CORES, b_total // N_CORES,
                                 *tokens.shape[1:]).astype(np.int32)
    weights = [jnp.asarray(np.asarray(inputs[k], dtype=np.float32))
               for k in WEIGHT_KEYS]
    out = _pmodel(jnp.asarray(tok_sharded), *weights)
    out = np.asarray(jax.device_get(out)).reshape(b_total, LBL)
    return out.astype(np.float32)

